# Initial kernel scaffold
#
"""Trainium2 Bass kernel for LogicDense (soft differentiable logic layer).

Computation: out[n, j] = c0[j] + c1[j]*a + c2[j]*b + c3[j]*a*b
  where a = x[n, idx0[j]], b = x[n, idx1[j]] and
  coeff[j] = softmax(weight[j]) @ T  (T = 16x4 logic-op coefficient table).

Strategy (8 NeuronCores, data-parallel over batch):
  - Each core owns 512 batch rows. Host passes x shard TRANSPOSED (in_dim,
    512) so the feature gather becomes a contiguous-row gather; the device
    uses GPSIMD dma_gather (MoE-style SWDGE gather: one 2 KiB row per
    gathered feature, row i -> partition i%128).
  - Gathered tiles are (128 out-cols, 512 batch). The polynomial is
    evaluated with per-partition scalar coefficients (DVE tensor_scalar +
    ACT activation), the a*(...) product on DVE, and the final add is folded
    into PSUM accumulation of two PE transposes (t^T + q^T accumulate in the
    same PSUM bank). ScalarE copies PSUM -> SBUF staging laid out so each
    DMA store writes 4 KiB contiguous runs of the (batch, out_dim) output.
  - softmax(weight)@[T|1] runs on device: Exp on ScalarE over weight^T
    (16, 8192), 64 tiny PE matmuls against the (16,5) table, normalization
    by the partial-sum reciprocal on DVE.
"""

import numpy as np

BATCH, IN_DIM, OUT_DIM = 4096, 4096, 8192
N_CORES = 8
BSH = BATCH // N_CORES      # 512 batch rows per core
NBLK = 1024                 # out-cols per gather super-block
NCHUNK = NBLK // 128        # 8 chunks (of 128 out-cols) per super-block
NSB = OUT_DIM // NBLK       # 8 super-blocks
SSUB = BSH // 128           # 4 batch sub-blocks of 128
NT = OUT_DIM // 128         # 64 coefficient blocks

# difflogic bin_op_s coefficient table: op_i(a,b) = T[i,0] + T[i,1]*a +
# T[i,2]*b + T[i,3]*a*b
_T = np.array([
    [0.0,  0.0,  0.0,  0.0],
    [0.0,  0.0,  0.0,  1.0],
    [0.0,  1.0,  0.0, -1.0],
    [0.0,  1.0,  0.0,  0.0],
    [0.0,  0.0,  1.0, -1.0],
    [0.0,  0.0,  1.0,  0.0],
    [0.0,  1.0,  1.0, -2.0],
    [0.0,  1.0,  1.0, -1.0],
    [1.0, -1.0, -1.0,  1.0],
    [1.0, -1.0, -1.0,  2.0],
    [1.0,  0.0, -1.0,  0.0],
    [1.0,  0.0, -1.0,  1.0],
    [1.0, -1.0,  0.0,  0.0],
    [1.0, -1.0,  0.0,  1.0],
    [1.0,  0.0,  0.0, -1.0],
    [1.0,  0.0,  0.0,  0.0],
], dtype=np.float32)

_CACHE = {}


def build_program(repeat=None):
    """Build + compile the per-core Bass program (cached per process).

    repeat=K wraps the main gather/compute/store loop in a device-side
    For_i loop that runs it K times — used only for timing (the work is
    idempotent), never for the real kernel() path.
    """
    key = ("nc", repeat)
    if key in _CACHE:
        return _CACHE[key]

    import concourse.tile as tile
    import concourse.mybir as mybir
    from concourse import bacc

    dt = mybir.dt
    f32 = dt.float32
    Alu = mybir.AluOpType
    Act = mybir.ActivationFunctionType

    nc = bacc.Bacc("TRN2", target_bir_lowering=False, debug=False,
                   num_devices=N_CORES)
    xT = nc.dram_tensor("xT", [IN_DIM, BSH], f32, kind="ExternalInput").ap()
    idxa = nc.dram_tensor("idxa", [128, OUT_DIM // 16], dt.int16,
                          kind="ExternalInput").ap()
    idxb = nc.dram_tensor("idxb", [128, OUT_DIM // 16], dt.int16,
                          kind="ExternalInput").ap()
    wT = nc.dram_tensor("wT", [16, OUT_DIM], f32, kind="ExternalInput").ap()
    tmat = nc.dram_tensor("tmat", [16, 5], f32, kind="ExternalInput").ap()
    ident = nc.dram_tensor("ident", [128, 128], f32,
                           kind="ExternalInput").ap()
    out = nc.dram_tensor("out", [BSH, OUT_DIM], f32,
                         kind="ExternalOutput").ap()
    # (s p) rows, (g n) cols -> per-super-block store view
    out_r = out.rearrange("(s p) (g n) -> p s g n", p=128, n=NBLK)

    with tile.TileContext(nc) as tc:
        with (
            tc.tile_pool(name="const", bufs=1) as constp,
            tc.tile_pool(name="coef", bufs=1) as cpool,
            tc.tile_pool(name="gather", bufs=2) as gpool,
            tc.tile_pool(name="stage", bufs=2) as spool,
            tc.tile_pool(name="tmp", bufs=3) as tpool,
            tc.tile_pool(name="po", bufs=4, space="PSUM") as pspool,
            tc.tile_pool(name="pu", bufs=2, space="PSUM") as pupool,
        ):
            idxa_sb = constp.tile([128, OUT_DIM // 16], dt.int16)
            nc.sync.dma_start(idxa_sb[:], idxa)
            idxb_sb = constp.tile([128, OUT_DIM // 16], dt.int16)
            nc.sync.dma_start(idxb_sb[:], idxb)
            wT_sb = constp.tile([16, OUT_DIM], f32)
            nc.sync.dma_start(wT_sb[:], wT)
            tmat_sb = constp.tile([16, 5], f32)
            nc.sync.dma_start(tmat_sb[:], tmat)
            ident_sb = constp.tile([128, 128], f32)
            nc.sync.dma_start(ident_sb[:], ident)

            # --- coefficients: u = exp(w^T).T @ [T|1]; cnorm = u[:, :4]/u[:, 4]
            expw = cpool.tile([16, OUT_DIM], f32)
            nc.scalar.activation(expw[:], wT_sb[:], Act.Exp)
            u_all = cpool.tile([128, NT, 5], f32)
            for t in range(NT):
                pu = pupool.tile([128, 5], f32)
                nc.tensor.matmul(pu[:], expw[:, t * 128:(t + 1) * 128],
                                 tmat_sb[:], start=True, stop=True)
                nc.scalar.activation(u_all[:, t, :], pu[:], Act.Copy)
            rcp = cpool.tile([128, NT], f32)
            nc.vector.reciprocal(rcp[:], u_all[:, :, 4])
            cnorm = cpool.tile([128, NT, 4], f32)
            for k in range(4):
                nc.vector.tensor_tensor(cnorm[:, :, k], u_all[:, :, k],
                                        rcp[:], Alu.mult)

            # --- main loop over 8 super-blocks of 1024 out-cols
            def main_loop():
                for g in range(NSB):
                    main_block(g)

            def main_block(g):
                ga = gpool.tile([128, NCHUNK, BSH], f32, tag="ga")
                gb = gpool.tile([128, NCHUNK, BSH], f32, tag="gb")
                nc.gpsimd.dma_gather(ga[:], xT,
                                     idxa_sb[:, g * 64:(g + 1) * 64],
                                     NBLK, NBLK, BSH)
                nc.gpsimd.dma_gather(gb[:], xT,
                                     idxb_sb[:, g * 64:(g + 1) * 64],
                                     NBLK, NBLK, BSH)
                stage = spool.tile([128, SSUB, NBLK], f32, tag="stage")
                for c in range(NCHUNK):
                    tb = g * NCHUNK + c
                    a = ga[:, c, :]
                    b = gb[:, c, :]
                    c0 = cnorm[:, tb, 0:1]
                    c1 = cnorm[:, tb, 1:2]
                    c2 = cnorm[:, tb, 2:3]
                    c3 = cnorm[:, tb, 3:4]
                    # p = c3*b + c1   (DVE, fp32 tensor_scalar runs 2x)
                    p = tpool.tile([128, BSH], f32, tag="p")
                    nc.vector.tensor_scalar(p[:], b, c3, c1, Alu.mult,
                                            Alu.add)
                    # q = c2*b + c0   (ScalarE)
                    q = tpool.tile([128, BSH], f32, tag="q")
                    nc.scalar.activation(q[:], b, Act.Identity, bias=c0,
                                         scale=c2)
                    # t = p * a       (DVE)
                    tt = tpool.tile([128, BSH], f32, tag="t")
                    nc.vector.tensor_tensor(tt[:], p[:], a, Alu.mult)
                    # out^T = t^T + q^T via PE transpose w/ PSUM accumulate
                    po = pspool.tile([128, SSUB, 128], f32)
                    for s in range(SSUB):
                        nc.tensor.matmul(po[:, s, :],
                                         tt[:, s * 128:(s + 1) * 128],
                                         ident_sb[:], is_transpose=True,
                                         start=True, stop=False)
                        nc.tensor.matmul(po[:, s, :],
                                         q[:, s * 128:(s + 1) * 128],
                                         ident_sb[:], is_transpose=True,
                                         start=False, stop=True)
                    nc.scalar.activation(stage[:, :, c * 128:(c + 1) * 128],
                                         po[:], Act.Copy)
                nc.sync.dma_start(out_r[:, :, g, :], stage[:])

            if repeat is None:
                main_loop()
            else:
                with tc.For_i(0, repeat, 1):
                    main_loop()

    nc.compile()
    _CACHE[key] = nc
    return nc


def _wrap_idxs(idx):
    """int64 (OUT_DIM,) -> SWDGE-wrapped int16 (128, OUT_DIM//16) table.

    Per 1024-idx super-block g, idx i lives at [p, g*64 + i//16] for
    p % 16 == i % 16 (replicated across the eight 16-partition groups).
    """
    tab = np.empty((128, OUT_DIM // 16), np.int16)
    for g in range(NSB):
        blk = idx[g * NBLK:(g + 1) * NBLK].astype(np.int16)
        w = blk.reshape(64, 16).T            # [p16, s64]
        tab[:, g * 64:(g + 1) * 64] = np.tile(w, (8, 1))
    return tab


def prepare_in_maps(x, indices, weight):
    x = np.asarray(x, np.float32)
    indices = np.asarray(indices)
    weight = np.asarray(weight, np.float32)
    ia = _wrap_idxs(indices[0])
    ib = _wrap_idxs(indices[1])
    wTm = np.ascontiguousarray(weight.T)
    tm = np.ascontiguousarray(np.concatenate(
        [_T, np.ones((16, 1), np.float32)], axis=1))
    idm = np.eye(128, dtype=np.float32)
    xT = x.T
    in_maps = []
    for c in range(N_CORES):
        in_maps.append({
            "xT": np.ascontiguousarray(xT[:, c * BSH:(c + 1) * BSH]),
            "idxa": ia, "idxb": ib, "wT": wTm, "tmat": tm, "ident": idm,
        })
    return in_maps


def kernel(x, indices, weight):
    from concourse.bass_utils import run_bass_kernel_spmd

    nc = build_program()
    in_maps = prepare_in_maps(x, indices, weight)
    res = run_bass_kernel_spmd(nc, in_maps, core_ids=list(range(N_CORES)))
    full = np.empty((BATCH, OUT_DIM), np.float32)
    for c in range(N_CORES):
        full[c * BSH:(c + 1) * BSH] = res.results[c]["out"]
    return full



# revision 1
# speedup vs baseline: 1.6937x; 1.6937x over previous
"""Trainium2 Bass kernel for LogicDense (soft differentiable logic layer).

Computation: out[n, j] = c0[j] + c1[j]*a + c2[j]*b + c3[j]*a*b
  where a = x[n, idx0[j]], b = x[n, idx1[j]] and
  coeff[j] = softmax(weight[j]) @ T  (T = 16x4 logic-op coefficient table).

Strategy (8 NeuronCores, data-parallel over batch):
  - Each core owns 512 batch rows. Host passes x shard TRANSPOSED (in_dim,
    512) so the feature gather becomes a contiguous-row gather; the device
    uses GPSIMD dma_gather (MoE-style SWDGE gather: one 2 KiB row per
    gathered feature, row i -> partition i%128).
  - Gathered tiles are (128 out-cols, 512 batch). The polynomial is
    evaluated with per-partition scalar coefficients (DVE tensor_scalar +
    ACT activation), the a*(...) product on DVE, and the final add is folded
    into PSUM accumulation of two PE transposes (t^T + q^T accumulate in the
    same PSUM bank). ScalarE copies PSUM -> SBUF staging laid out so each
    DMA store writes 4 KiB contiguous runs of the (batch, out_dim) output.
  - softmax(weight)@[T|1] runs on device: Exp on ScalarE over weight^T
    (16, 8192), 64 tiny PE matmuls against the (16,5) table, normalization
    by the partial-sum reciprocal on DVE.
"""

import numpy as np

BATCH, IN_DIM, OUT_DIM = 4096, 4096, 8192
N_CORES = 8
BSH = BATCH // N_CORES      # 512 batch rows per core
NBLK = 1024                 # out-cols per gather super-block
NCHUNK = NBLK // 128        # 8 chunks (of 128 out-cols) per super-block
NSB = OUT_DIM // NBLK       # 8 super-blocks
SSUB = BSH // 128           # 4 batch sub-blocks of 128
NT = OUT_DIM // 128         # 64 coefficient blocks

# difflogic bin_op_s coefficient table: op_i(a,b) = T[i,0] + T[i,1]*a +
# T[i,2]*b + T[i,3]*a*b
_T = np.array([
    [0.0,  0.0,  0.0,  0.0],
    [0.0,  0.0,  0.0,  1.0],
    [0.0,  1.0,  0.0, -1.0],
    [0.0,  1.0,  0.0,  0.0],
    [0.0,  0.0,  1.0, -1.0],
    [0.0,  0.0,  1.0,  0.0],
    [0.0,  1.0,  1.0, -2.0],
    [0.0,  1.0,  1.0, -1.0],
    [1.0, -1.0, -1.0,  1.0],
    [1.0, -1.0, -1.0,  2.0],
    [1.0,  0.0, -1.0,  0.0],
    [1.0,  0.0, -1.0,  1.0],
    [1.0, -1.0,  0.0,  0.0],
    [1.0, -1.0,  0.0,  1.0],
    [1.0,  0.0,  0.0, -1.0],
    [1.0,  0.0,  0.0,  0.0],
], dtype=np.float32)

_CACHE = {}


def build_program(repeat=None):
    """Build + compile the per-core Bass program (cached per process).

    repeat=K wraps the main gather/compute/store loop in a device-side
    For_i loop that runs it K times — used only for timing (the work is
    idempotent), never for the real kernel() path.
    """
    key = ("nc", repeat)
    if key in _CACHE:
        return _CACHE[key]

    import concourse.tile as tile
    import concourse.mybir as mybir
    from concourse import bacc

    dt = mybir.dt
    f32 = dt.float32
    Alu = mybir.AluOpType
    Act = mybir.ActivationFunctionType

    nc = bacc.Bacc("TRN2", target_bir_lowering=False, debug=False,
                   num_devices=N_CORES)
    xT = nc.dram_tensor("xT", [IN_DIM, BSH], f32, kind="ExternalInput").ap()
    idxa = nc.dram_tensor("idxa", [128, OUT_DIM // 16], dt.int16,
                          kind="ExternalInput").ap()
    idxb = nc.dram_tensor("idxb", [128, OUT_DIM // 16], dt.int16,
                          kind="ExternalInput").ap()
    wT = nc.dram_tensor("wT", [16, OUT_DIM], f32, kind="ExternalInput").ap()
    tmat = nc.dram_tensor("tmat", [16, 5], f32, kind="ExternalInput").ap()
    ident = nc.dram_tensor("ident", [128, 128], f32,
                           kind="ExternalInput").ap()
    out = nc.dram_tensor("out", [BSH, OUT_DIM], f32,
                         kind="ExternalOutput").ap()
    # (s p) rows, (g n) cols -> per-super-block store view
    out_r = out.rearrange("(s p) (g n) -> p s g n", p=128, n=NBLK)

    with tile.TileContext(nc) as tc:
        with (
            tc.tile_pool(name="const", bufs=1) as constp,
            tc.tile_pool(name="coef", bufs=1) as cpool,
            tc.tile_pool(name="gather", bufs=2) as gpool,
            tc.tile_pool(name="stage", bufs=2) as spool,
            tc.tile_pool(name="tmp", bufs=3) as tpool,
            tc.tile_pool(name="po", bufs=4, space="PSUM") as pspool,
            tc.tile_pool(name="pu", bufs=2, space="PSUM") as pupool,
        ):
            idxa_sb = constp.tile([128, OUT_DIM // 16], dt.int16)
            nc.sync.dma_start(idxa_sb[:], idxa)
            idxb_sb = constp.tile([128, OUT_DIM // 16], dt.int16)
            nc.sync.dma_start(idxb_sb[:], idxb)
            wT_sb = constp.tile([16, OUT_DIM], f32)
            nc.sync.dma_start(wT_sb[:], wT)
            tmat_sb = constp.tile([16, 5], f32)
            nc.sync.dma_start(tmat_sb[:], tmat)
            ident_sb = constp.tile([128, 128], f32)
            nc.sync.dma_start(ident_sb[:], ident)

            # --- coefficients: u = exp(w^T).T @ [T|1]; cnorm = u[:, :4]/u[:, 4]
            expw = cpool.tile([16, OUT_DIM], f32)
            nc.scalar.activation(expw[:], wT_sb[:], Act.Exp)
            u_all = cpool.tile([128, NT, 5], f32)
            for t in range(NT):
                pu = pupool.tile([128, 5], f32)
                nc.tensor.matmul(pu[:], expw[:, t * 128:(t + 1) * 128],
                                 tmat_sb[:], start=True, stop=True)
                nc.scalar.activation(u_all[:, t, :], pu[:], Act.Copy)
            rcp = cpool.tile([128, NT], f32)
            nc.vector.reciprocal(rcp[:], u_all[:, :, 4])
            cnorm = cpool.tile([128, NT, 4], f32)
            for k in range(4):
                nc.vector.tensor_tensor(cnorm[:, :, k], u_all[:, :, k],
                                        rcp[:], Alu.mult)

            # --- main loop over 8 super-blocks of 1024 out-cols
            def main_loop():
                for g in range(NSB):
                    main_block(g)

            def main_block(g):
                ga = gpool.tile([128, NCHUNK, BSH], f32, tag="ga")
                gb = gpool.tile([128, NCHUNK, BSH], f32, tag="gb")
                nc.gpsimd.dma_gather(ga[:], xT,
                                     idxa_sb[:, g * 64:(g + 1) * 64],
                                     NBLK, NBLK, BSH)
                nc.gpsimd.dma_gather(gb[:], xT,
                                     idxb_sb[:, g * 64:(g + 1) * 64],
                                     NBLK, NBLK, BSH)
                stage = spool.tile([128, SSUB, NBLK], f32, tag="stage")
                for c in range(NCHUNK):
                    tb = g * NCHUNK + c
                    a = ga[:, c, :]
                    b = gb[:, c, :]
                    c0 = cnorm[:, tb, 0:1]
                    c1 = cnorm[:, tb, 1:2]
                    c2 = cnorm[:, tb, 2:3]
                    c3 = cnorm[:, tb, 3:4]
                    # p = c3*b + c1   (DVE, fp32 tensor_scalar runs 2x)
                    p = tpool.tile([128, BSH], f32, tag="p")
                    nc.vector.tensor_scalar(p[:], b, c3, c1, Alu.mult,
                                            Alu.add)
                    # q = c2*b + c0   (ScalarE)
                    q = tpool.tile([128, BSH], f32, tag="q")
                    nc.scalar.activation(q[:], b, Act.Identity, bias=c0,
                                         scale=c2)
                    # t = p * a       (DVE)
                    tt = tpool.tile([128, BSH], f32, tag="t")
                    nc.vector.tensor_tensor(tt[:], p[:], a, Alu.mult)
                    # out^T = t^T + q^T via PE transpose w/ PSUM accumulate
                    po = pspool.tile([128, SSUB, 128], f32)
                    for s in range(SSUB):
                        nc.tensor.matmul(po[:, s, :],
                                         tt[:, s * 128:(s + 1) * 128],
                                         ident_sb[:], is_transpose=True,
                                         start=True, stop=False)
                        nc.tensor.matmul(po[:, s, :],
                                         q[:, s * 128:(s + 1) * 128],
                                         ident_sb[:], is_transpose=True,
                                         start=False, stop=True)
                    nc.scalar.activation(stage[:, :, c * 128:(c + 1) * 128],
                                         po[:], Act.Copy)
                nc.sync.dma_start(out_r[:, :, g, :], stage[:])

            if repeat is None:
                main_loop()
            else:
                with tc.For_i(0, repeat, 1):
                    main_loop()

    nc.compile()
    _CACHE[key] = nc
    return nc


def _wrap_idxs(idx):
    """int64 (OUT_DIM,) -> SWDGE-wrapped int16 (128, OUT_DIM//16) table.

    Per 1024-idx super-block g, idx i lives at [p, g*64 + i//16] for
    p % 16 == i % 16 (replicated across the eight 16-partition groups).
    """
    tab = np.empty((128, OUT_DIM // 16), np.int16)
    for g in range(NSB):
        blk = idx[g * NBLK:(g + 1) * NBLK].astype(np.int16)
        w = blk.reshape(64, 16).T            # [p16, s64]
        tab[:, g * 64:(g + 1) * 64] = np.tile(w, (8, 1))
    return tab


def prepare_in_maps(x, indices, weight):
    x = np.asarray(x, np.float32)
    indices = np.asarray(indices)
    weight = np.asarray(weight, np.float32)
    ia = _wrap_idxs(indices[0])
    ib = _wrap_idxs(indices[1])
    wTm = np.ascontiguousarray(weight.T)
    tm = np.ascontiguousarray(np.concatenate(
        [_T, np.ones((16, 1), np.float32)], axis=1))
    idm = np.eye(128, dtype=np.float32)
    xT = x.T
    in_maps = []
    for c in range(N_CORES):
        in_maps.append({
            "xT": np.ascontiguousarray(xT[:, c * BSH:(c + 1) * BSH]),
            "idxa": ia, "idxb": ib, "wT": wTm, "tmat": tm, "ident": idm,
        })
    return in_maps


def kernel(x, indices, weight):
    from concourse.bass_utils import run_bass_kernel_spmd

    nc = build_program()
    in_maps = prepare_in_maps(x, indices, weight)
    res = run_bass_kernel_spmd(nc, in_maps, core_ids=list(range(N_CORES)))
    full = np.empty((BATCH, OUT_DIM), np.float32)
    for c in range(N_CORES):
        full[c * BSH:(c + 1) * BSH] = res.results[c]["out"]
    return full

